# revision 9
# baseline (speedup 1.0000x reference)
"""Bass/Trainium2 kernel for nn_Attn (dot+affect attention over encoder outputs).

Computation (per batch b):
  e[b, l] = h[b] . enc[l, b]  +  (h[b] @ affect) . emb[l, b]
  out[b, 0, :] = softmax(e[b, :])

Strategy: data-parallel over batch (8 batches per core on 8 cores). The host
packs enc and emb into one [L, BLOC*(H+A)] tensor per core so wide elementwise
multiplies + free-dim reductions compute the full dot product in one pass over
the streamed data (memory-bound regime). Engine balance per 4.2MB slab:
VectorE multiplies 6 batches in place, GpSimd 2; reductions go 2 to VectorE
(tensor_reduce) and 6 to ScalarE (activation Copy with accumulate). Softmax
runs on a transposed [128, 128] score tile using mask matmuls for
partition-group reductions/broadcasts.
"""

import numpy as np

import concourse.bass as bass
import concourse.tile as tile
from concourse import bacc, mybir
from concourse.bass_utils import run_bass_kernel_spmd

F32 = mybir.dt.float32
L, B, H, A = 2048, 64, 1024, 3
NCORES = 8
BLOC = B // NCORES          # batches per core
HE = H + A                  # extended hidden width (dot + affect features)
P = 128                     # SBUF partitions / l-tile height

N_GPS = 2                   # batches whose multiply runs on GpSimd
N_DVE_RED = 2               # batches whose reduction runs on VectorE


def build_nc(l_total: int = L):
    no = l_total // P       # number of l-tiles
    cols = BLOC * no        # score columns: c = b*no + o

    nc = bacc.Bacc("TRN2", target_bir_lowering=False, debug=False)

    enc_d = nc.dram_tensor("enc", [l_total, BLOC * HE], F32, kind="ExternalInput")
    hid_d = nc.dram_tensor("hid", [BLOC, H], F32, kind="ExternalInput")
    aff_d = nc.dram_tensor("aff", [1, H * A], F32, kind="ExternalInput")
    ident_d = nc.dram_tensor("ident", [P, P], F32, kind="ExternalInput")
    ones_d = nc.dram_tensor("ones_", [1, P], F32, kind="ExternalInput")
    bm_d = nc.dram_tensor("bm", [cols, BLOC], F32, kind="ExternalInput")
    bmT_d = nc.dram_tensor("bmT", [BLOC, cols], F32, kind="ExternalInput")
    nbmT_d = nc.dram_tensor("nbmT", [BLOC, cols], F32, kind="ExternalInput")
    sel_d = nc.dram_tensor("sel", [BLOC, BLOC * P], F32, kind="ExternalInput")
    out_d = nc.dram_tensor("out", [BLOC, l_total], F32, kind="ExternalOutput")

    add = mybir.AluOpType.add
    amax = mybir.AluOpType.max
    AX = mybir.AxisListType.X
    Copy = mybir.ActivationFunctionType.Copy
    Exp = mybir.ActivationFunctionType.Exp

    with tile.TileContext(nc) as tc:
        with (
            tc.tile_pool(name="const", bufs=1) as cpool,
            tc.tile_pool(name="slab", bufs=3) as spool,
            tc.tile_pool(name="scratch", bufs=2) as tpool,
            tc.tile_pool(name="ps_bc", bufs=2, space="PSUM") as ppool,
            tc.tile_pool(name="ps_sm", bufs=4, space="PSUM") as qpool,
        ):
            # ---- constants / small inputs (gpsimd DMA queue: keep the sync
            # queue free for the big streaming slabs) ----
            ident = cpool.tile([P, P], F32)
            nc.gpsimd.dma_start(ident[:], ident_d[:])
            ones = cpool.tile([1, P], F32)
            nc.gpsimd.dma_start(ones[:], ones_d[:])
            bm = cpool.tile([cols, BLOC], F32)
            nc.gpsimd.dma_start(bm[:], bm_d[:])
            bmT = cpool.tile([BLOC, cols], F32)
            nc.gpsimd.dma_start(bmT[:], bmT_d[:])
            nbmT = cpool.tile([BLOC, cols], F32)
            nc.gpsimd.dma_start(nbmT[:], nbmT_d[:])
            h_sb = cpool.tile([BLOC, H], F32)
            nc.gpsimd.dma_start(h_sb[:], hid_d[:])
            sel = cpool.tile([BLOC, BLOC * P], F32)
            nc.gpsimd.dma_start(sel[:], sel_d[:])

            # ---- ha = h @ affect  ([BLOC, A]) ----
            # DMA affect column k to partition 0, broadcast it over BLOC
            # partitions with a K=1 matmul, multiply with h on VectorE
            # (reading PSUM), reduce on ScalarE.
            aff_r = aff_d[:].rearrange("p (h k) -> p k h", k=A)   # [1, A, H]
            ha_sb = cpool.tile([BLOC, A], F32)
            for k in range(A):
                af_row = tpool.tile([1, H], F32, tag="afrow", name="af_row")
                nc.gpsimd.dma_start(af_row[:], aff_r[:, k, :])
                af_ps = ppool.tile([BLOC, H], F32, tag="bc", name="af_ps")
                for j in range(H // 512):
                    nc.tensor.matmul(
                        af_ps[:, bass.ts(j, 512)],
                        ones[0:1, 0:BLOC],
                        af_row[0:1, bass.ts(j, 512)],
                        start=True, stop=True,
                    )
                scr_h = tpool.tile([BLOC, H], F32, tag="cpy", name="scr_h")
                nc.vector.tensor_mul(scr_h[:], h_sb[:], af_ps[:])
                scr_c = tpool.tile([BLOC, H], F32, tag="cpy", name="scr_c")
                nc.scalar.activation(scr_c[:], scr_h[:], Copy,
                                     accum_out=ha_sb[:, k:k + 1])

            # ---- hbext: per-b extended hidden [h[b], ha[b]] broadcast to 128
            # partitions. sel[:, b*P:(b+1)*P] is a one-hot row-b selector: the
            # matmul selects row b and broadcasts it from base partition 0. ----
            hbext = cpool.tile([P, BLOC * HE], F32)
            for b in range(BLOC):
                eb = sel[:, bass.ts(b, P)]
                for j in range(H // 512):
                    hb_ps = ppool.tile([P, 512], F32, tag="bc", name="hb_ps")
                    nc.tensor.matmul(hb_ps[:], eb, h_sb[:, bass.ts(j, 512)],
                                     start=True, stop=True)
                    dst = hbext[:, b * HE + j * 512:b * HE + (j + 1) * 512]
                    if b % 2 == 0:
                        nc.scalar.copy(dst, hb_ps[:])
                    else:
                        nc.vector.tensor_copy(dst, hb_ps[:])
                hab_ps = ppool.tile([P, A], F32, tag="bc", name="hab_ps")
                nc.tensor.matmul(hab_ps[:], eb, ha_sb[:], start=True, stop=True)
                nc.scalar.copy(hbext[:, b * HE + H:b * HE + H + A], hab_ps[:])

            # ---- main loop: stream enc slabs, in-place multiply, reduce ----
            scores = cpool.tile([P, cols], F32)
            enc_r = enc_d[:].rearrange("(o p) f -> o p f", p=P)   # [no, P, BLOC*HE]
            n_dve_mul = BLOC - N_GPS
            for o in range(no):
                slab = spool.tile([P, BLOC * HE], F32, tag="slab", name="slab")
                nc.sync.dma_start(slab[:], enc_r[o])
                # in-place multiplies (slab *= hbext)
                nc.vector.tensor_mul(
                    slab[:, 0:n_dve_mul * HE],
                    slab[:, 0:n_dve_mul * HE],
                    hbext[:, 0:n_dve_mul * HE],
                )
                nc.gpsimd.tensor_mul(
                    slab[:, n_dve_mul * HE:BLOC * HE],
                    slab[:, n_dve_mul * HE:BLOC * HE],
                    hbext[:, n_dve_mul * HE:BLOC * HE],
                )
                # per-batch reductions into score columns
                for b in range(BLOC):
                    c = b * no + o
                    seg = slab[:, b * HE:(b + 1) * HE]
                    if b < N_DVE_RED:
                        nc.vector.tensor_reduce(scores[:, c:c + 1], seg,
                                                axis=AX, op=add)
                    else:
                        cpy = tpool.tile([P, HE], F32, tag="cpy", name="cpy")
                        nc.scalar.activation(cpy[:], seg, Copy,
                                             accum_out=scores[:, c:c + 1])

            # ---- softmax over l per batch, on transposed scores ----
            scT_ps = qpool.tile([cols, P], F32, tag="sm", name="scT_ps")
            nc.tensor.transpose(scT_ps[:], scores[:], ident[:])
            scT = cpool.tile([cols, P], F32)
            nc.scalar.copy(scT[:], scT_ps[:])

            rowmax = cpool.tile([cols, 1], F32)
            nc.vector.tensor_reduce(rowmax[:], scT[:], axis=AX, op=amax)
            rmT_ps = qpool.tile([1, cols], F32, tag="sm", name="rmT_ps")
            nc.tensor.matmul(rmT_ps[:], rowmax[:], ident[0:cols, 0:cols],
                             start=True, stop=True)
            rm_sb = cpool.tile([1, cols], F32)
            nc.scalar.copy(rm_sb[:], rmT_ps[:])
            bmax = cpool.tile([1, BLOC], F32)
            nc.vector.tensor_reduce(
                bmax[:], rm_sb[:].rearrange("p (b o) -> p b o", b=BLOC),
                axis=AX, op=amax)
            bcol_ps = qpool.tile([BLOC, 1], F32, tag="sm", name="bcol_ps")
            nc.tensor.matmul(bcol_ps[:], bmax[:], ones[0:1, 0:1],
                             start=True, stop=True)
            bcol = cpool.tile([BLOC, 1], F32)
            nc.scalar.copy(bcol[:], bcol_ps[:])
            negm_ps = qpool.tile([cols, 1], F32, tag="sm", name="negm_ps")
            nc.tensor.matmul(negm_ps[:], nbmT[:], bcol[:], start=True, stop=True)
            negm = cpool.tile([cols, 1], F32)
            nc.scalar.copy(negm[:], negm_ps[:])

            expT = cpool.tile([cols, P], F32)
            rowsum = cpool.tile([cols, 1], F32)
            nc.scalar.activation(expT[:], scT[:], Exp, bias=negm[:], scale=1.0,
                                 accum_out=rowsum[:])
            ssum_ps = qpool.tile([BLOC, 1], F32, tag="sm", name="ssum_ps")
            nc.tensor.matmul(ssum_ps[:], bm[:], rowsum[:], start=True, stop=True)
            rsum = cpool.tile([BLOC, 1], F32)
            nc.vector.reciprocal(rsum[:], ssum_ps[:])
            rbc_ps = qpool.tile([cols, 1], F32, tag="sm", name="rbc_ps")
            nc.tensor.matmul(rbc_ps[:], bmT[:], rsum[:], start=True, stop=True)
            rbc = cpool.tile([cols, 1], F32)
            nc.scalar.copy(rbc[:], rbc_ps[:])

            outT = cpool.tile([cols, P], F32)
            nc.vector.tensor_scalar_mul(outT[:], expT[:], rbc[:, 0:1])
            nc.sync.dma_start(out_d[:].rearrange("b (o li) -> (b o) li", o=no),
                              outT[:])

    nc.compile()
    return nc


def make_aux(l_total: int = L):
    no = l_total // P
    cols = BLOC * no
    ident = np.eye(P, dtype=np.float32)
    ones_ = np.ones((1, P), dtype=np.float32)
    bmT = np.zeros((BLOC, cols), dtype=np.float32)
    for b in range(BLOC):
        bmT[b, b * no:(b + 1) * no] = 1.0
    sel = np.zeros((BLOC, BLOC * P), dtype=np.float32)
    for b in range(BLOC):
        sel[b, b * P:(b + 1) * P] = 1.0
    return {
        "ident": ident,
        "ones_": ones_,
        "bm": np.ascontiguousarray(bmT.T),
        "bmT": bmT,
        "nbmT": -bmT,
        "sel": sel,
    }


def make_in_maps(hidden, encoder_outputs, embedding, affect_matrix, l_total: int = L):
    aux = make_aux(l_total)
    aff = np.ascontiguousarray(affect_matrix.reshape(1, H * A), dtype=np.float32)
    in_maps = []
    for i in range(NCORES):
        bs = slice(i * BLOC, (i + 1) * BLOC)
        enc_ext = np.concatenate(
            [encoder_outputs[:, bs, :], embedding[:, bs, :]], axis=2
        ).reshape(l_total, BLOC * HE)
        in_maps.append({
            "enc": np.ascontiguousarray(enc_ext, dtype=np.float32),
            "hid": np.ascontiguousarray(hidden[0, bs, :], dtype=np.float32),
            "aff": aff,
            **aux,
        })
    return in_maps


_NC_CACHE = {}


def kernel(hidden, encoder_outputs, embedding, affect_matrix):
    hidden = np.asarray(hidden, dtype=np.float32)
    encoder_outputs = np.asarray(encoder_outputs, dtype=np.float32)
    embedding = np.asarray(embedding, dtype=np.float32)
    affect_matrix = np.asarray(affect_matrix, dtype=np.float32)

    if L not in _NC_CACHE:
        _NC_CACHE[L] = build_nc(L)
    nc = _NC_CACHE[L]
    in_maps = make_in_maps(hidden, encoder_outputs, embedding, affect_matrix, L)
    res = run_bass_kernel_spmd(nc, in_maps, list(range(NCORES))).results
    out = np.concatenate(
        [res[i]["out"].reshape(BLOC, 1, L) for i in range(NCORES)], axis=0
    )
    return out
